# revision 41
# baseline (speedup 1.0000x reference)
"""Multi-head causal attention (B=2, T=2048, C=1024, H=16, Dh=64) on 8 TRN2 cores.

Sharding: batch x head tensor-parallel. Core i handles batch i//4 and heads
4*(i%4) .. 4*(i%4)+3. All weights and x are pre-transposed / fp16-converted on
the host, so the device does no layout work:
  1. per 512-token chunk, load xT columns and project qT/kT/v directly in the
     feature-on-partition layout (fp16 matmuls, fp32 PSUM),
  2. causal flash attention in scoresT (keys x tokens) layout; softmax
     denominators via a ones-column folded into v'; odd heads carry the ones
     column first so their denominator lands at partition 63 and their y rows
     at 64..127 -- every normalize op stays partition-aligned and two heads
     pack one [128, 512] tile,
  3. partial output projection over the core's own 256 y-features into all
     1024 output features (+bias on group-rank-0 cores, zeros elsewhere),
  4. per-chunk ReduceScatter (sum) over the 4 cores of the batch: each core
     receives its 256 output-feature rows, already fully reduced.
Host reassembles (concat feature shards per chunk, transpose to token-major).
"""

import json

import numpy as np

import concourse.bass as bass
import concourse.mybir as mybir
from concourse.tile import TileContext
from concourse.bass_utils import run_bass_kernel_spmd
from concourse.masks import make_identity, make_upper_triangular

F32 = mybir.dt.float32
F32R = mybir.dt.float32r
F16 = mybir.dt.float16

N_CORES = 8
B = 2
T = 2048          # tokens per batch (= per core)
C = 1024          # model dim
NH_CORE = 4       # heads per core
DH = 64
FEATS = NH_CORE * DH   # 256 per-core q/k/v features
CCH = 512         # attention t-chunk
NCH = T // CCH    # 4 chunks
KTILES = T // 128  # 16 k-tiles
SCALE = 1.0 / 8.0  # 1/sqrt(DH)


def _split_waits_in_bir(bir_bytes: bytes) -> bytes:
    """Workaround: installed walrus rejects >1 sync-wait per instruction."""
    bir = json.loads(bir_bytes)
    changed = False

    def rewrite(insts):
        nonlocal changed
        out = []
        for inst in insts:
            if isinstance(inst, dict):
                for v in inst.values():
                    visit(v)
                si = inst.get("sync_info")
                engine = inst.get("engine")
                if si and engine and len(si.get("on_wait") or []) > 1:
                    waits = si["on_wait"]
                    for i, w in enumerate(waits[:-1]):
                        out.append(
                            {
                                "debug": inst.get("debug", 0),
                                "engine": engine,
                                "ins": [],
                                "name": f"{inst['name']}_ws{i}",
                                "opcode": "EventSemaphore",
                                "outs": [],
                                "sync_info": {"on_update": [], "on_wait": [w]},
                            }
                        )
                    si["on_wait"] = [waits[-1]]
                    changed = True
            out.append(inst)
        insts[:] = out

    def visit(o):
        if isinstance(o, dict):
            for k, v in o.items():
                if k == "instructions" and isinstance(v, list):
                    rewrite(v)
                else:
                    visit(v)
        elif isinstance(o, list):
            for v in o:
                visit(v)

    visit(bir)
    return json.dumps(bir).encode() if changed else bir_bytes


_PATCHED = False


def _apply_walrus_workaround():
    global _PATCHED
    if _PATCHED:
        return
    import concourse.bass_utils as bass_utils
    import concourse.bass2jax as bass2jax

    orig = bass_utils.compile_bir_kernel

    def wrapped(bir_json, tmpdir, neff_name="file.neff"):
        return orig(_split_waits_in_bir(bir_json), tmpdir, neff_name)

    bass_utils.compile_bir_kernel = wrapped
    bass2jax.compile_bir_kernel = wrapped
    _PATCHED = True


def _build_program() -> bass.Bass:
    nc = bass.Bass(num_devices=N_CORES)

    xT = nc.dram_tensor("xT", [C, T], F16, kind="ExternalInput")
    # weight tensors arrive in SBUF layout: [128, 8*256] (col block k = wT
    # rows 128k..) / [128, 2*1024] (col block m = woT rows 128m..)
    wqT = nc.dram_tensor("wqT", [128, 8 * FEATS], F16, kind="ExternalInput")
    wkT = nc.dram_tensor("wkT", [128, 8 * FEATS], F16, kind="ExternalInput")
    wvT = nc.dram_tensor("wvT", [128, 8 * FEATS], F16, kind="ExternalInput")
    woT = nc.dram_tensor("woT", [128, 2 * C], F16, kind="ExternalInput")
    bo = nc.dram_tensor("bo", [128, 8], F32, kind="ExternalInput")

    partial = [nc.dram_tensor(f"partial{c}", [C, CCH], F16) for c in range(NCH)]
    outr = [nc.dram_tensor(f"outr{c}", [FEATS, CCH], F16) for c in range(NCH)]
    outc = [
        nc.dram_tensor(f"outc{c}", [FEATS, CCH], F16, kind="ExternalOutput")
        for c in range(NCH)
    ]
    groups = [[0, 1, 2, 3], [4, 5, 6, 7]]

    with TileContext(nc) as tc:
        with (
            tc.tile_pool(name="const", bufs=1) as cpool,
            tc.tile_pool(name="wts", bufs=1) as wpool,
            tc.tile_pool(name="xload", bufs=2) as xload,
            tc.tile_pool(name="qkv", bufs=1) as qkv,
            tc.tile_pool(name="qcur", bufs=2) as qcur,
            tc.tile_pool(name="vchunk", bufs=2) as vchunk,
            tc.tile_pool(name="expw", bufs=4) as expw,
            tc.tile_pool(name="norm", bufs=2) as norm,
            tc.tile_pool(name="ysbp", bufs=2) as ysbp,
            tc.tile_pool(name="osb", bufs=3) as osb,
            tc.tile_pool(name="pp", bufs=2, space="PSUM") as pp,
            tc.tile_pool(name="sp", bufs=2, space="PSUM") as sp,
            tc.tile_pool(name="yp", bufs=2, space="PSUM") as yp,
        ):
            # ---- startup loads, emitted in first-consumption order: the
            # modeled DMA device is serial, so transfer order ~= emission
            # order across the two HWDGE queues
            wsb = {}
            wsb["q"] = wpool.tile([128, 8 * FEATS], F16, name="w_q")
            xs_pre = []
            xb0t = [
                xload.tile([128, 4 * CCH], F16, name=f"xb0_{h}", tag=f"xb{h}")
                for h in range(2)
            ]
            for half in range(2):
                weng = nc.sync if half == 0 else nc.scalar
                xeng = nc.scalar if half == 0 else nc.sync
                weng.dma_start(
                    out=wsb["q"][:, 1024 * half : 1024 * (half + 1)],
                    in_=wqT[:, 1024 * half : 1024 * (half + 1)],
                )
                xeng.dma_start(
                    out=xb0t[half][:].rearrange("p (k f) -> p k f", f=CCH),
                    in_=xT.ap()[512 * half : 512 * (half + 1), 0:CCH]
                    .rearrange("(k p) f -> p k f", p=128),
                )
                for k in range(4):
                    xs_pre.append(xb0t[half][:, CCH * k : CCH * (k + 1)])
            for sec, wdram in (("k", wkT), ("v", wvT)):
                wt = wpool.tile([128, 8 * FEATS], F16, name=f"w_{sec}")
                for half in range(2):
                    eng = nc.sync if half == 0 else nc.scalar
                    eng.dma_start(
                        out=wt[:, 1024 * half : 1024 * (half + 1)],
                        in_=wdram[:, 1024 * half : 1024 * (half + 1)],
                    )
                wsb[sec] = wt
            wo_sb = wpool.tile([128, 2 * C], F16, name="w_o")
            for m in range(2):
                eng = nc.sync if m == 0 else nc.scalar
                eng.dma_start(
                    out=wo_sb[:, C * m : C * (m + 1)],
                    in_=woT[:, C * m : C * (m + 1)],
                )

            # ---- constants (needed from the v'-transpose / first exp on) ----
            identity = cpool.tile([128, 128], F16)
            make_identity(nc, identity[:])
            mask = cpool.tile([128, 128], F16)
            make_upper_triangular(nc, mask[:], val=1.0, diag=True)
            ones_r = cpool.tile([128, 64], F32R)
            nc.vector.memset(ones_r[:].bitcast(F32), 1.0)
            bias_sb = cpool.tile([128, 8], F32)
            nc.sync.dma_start(out=bias_sb[:], in_=bo[:, :])

            # ---- persistent activations ----
            kT = [qkv.tile([128, T], F16, name=f"kT_{m}") for m in range(2)]
            # v' tiles: [v | ones-col]; the ones row of the AV output is the
            # softmax denominator
            vp = {}
            for h in range(NH_CORE):
                for j in range(KTILES):
                    t = qkv.tile([128, DH + 1], F16, name=f"vp_{h}_{j}")
                    nc.vector.memset(t[:, DH : DH + 1], 1.0)
                    vp[h, j] = t

            def prefetch_x(n):
                t0 = CCH * n
                xss = []
                for half in range(2):
                    xb = xload.tile(
                        [128, 4 * CCH], F16, name=f"xb{n}_{half}", tag=f"xb{half}"
                    )
                    nc.scalar.dma_start(
                        out=xb[:].rearrange("p (k f) -> p k f", f=CCH),
                        in_=xT.ap()[512 * half : 512 * (half + 1), t0 : t0 + CCH]
                        .rearrange("(k p) f -> p k f", p=128),
                    )
                    for k in range(4):
                        xss.append(xb[:, CCH * k : CCH * (k + 1)])
                return xss

            def project_chunk(n, xss):
                """Project tokens [512n, 512n+512): qT/kT columns, v' tiles."""
                t0 = CCH * n
                qT = []
                vch = []
                for sec in ("q", "k", "v"):
                    for m in range(2):
                        ps = pp.tile([128, CCH], F32, name="projps", tag="pp")
                        for k in range(8):
                            nc.tensor.matmul(
                                ps[:],
                                wsb[sec][:, 256 * k + 128 * m : 256 * k + 128 * (m + 1)],
                                xss[k][:],
                                start=(k == 0),
                                stop=(k == 7),
                            )
                        if sec == "q":
                            qt = qkv.tile([128, CCH], F16, name=f"qT{n}_{m}")
                            nc.scalar.copy(out=qt[:], in_=ps[:])
                            qT.append(qt)
                        elif sec == "k":
                            nc.vector.tensor_copy(
                                out=kT[m][:, t0 : t0 + CCH], in_=ps[:]
                            )
                        else:
                            vc = vchunk.tile([128, CCH], F16, name=f"vc{n}_{m}", tag=f"vc{m}")
                            nc.vector.tensor_copy(out=vc[:], in_=ps[:])
                            vch.append(vc)
                for h in range(NH_CORE):
                    m, b_ = h // 2, h % 2
                    for jj in range(4):
                        j = 4 * n + jj
                        tp = pp.tile([128, DH], F16, name="vtr", tag="pp")
                        nc.tensor.matmul(
                            tp[:],
                            vch[m][64 * b_ : 64 * (b_ + 1), 128 * jj : 128 * (jj + 1)],
                            identity[64 * b_ : 64 * (b_ + 1), 64 * b_ : 64 * (b_ + 1)],
                            is_transpose=True,
                        )
                        nc.vector.tensor_copy(
                            out=vp[h, j][:, 0:DH], in_=tp[:]
                        )
                return qT

            def attend_chunk(c, qT):
                """Attention for tokens [512c, 512c+512), all heads + ysb."""
                jlast = 4 * c + 3
                ysb = [
                    ysbp.tile([128, CCH], F16, name=f"ysb{c}_{m}", tag=f"ysb{m}")
                    for m in range(2)
                ]
                # Heads run in interleaved PAIRS: while one head's exp is in
                # flight the PE streams the other head's matmuls, hiding the
                # score->exp->AV latency. Odd head leads (its ysb rows need a
                # partition-shifting SBUF->SBUF DMA, which then overlaps).
                def head_ctx(h):
                    m, b_ = h // 2, h % 2
                    ytp_t = yp.tile([128, CCH], F32, name=f"ytp{c}_{h}", tag="ytp")
                    return {
                        "h": h, "m": m, "b": b_,
                        "hq": qT[m][64 * b_ : 64 * (b_ + 1), :],
                        "hk": kT[m][64 * b_ : 64 * (b_ + 1), :],
                        "ytp_t": ytp_t,
                    }

                def attend_pair(ctx, p):
                    # two j-tiles share one PSUM score tile and one exp
                    # instruction: halves the Act engine's per-instruction
                    # access-latency overhead, which paces the pipeline
                    sc = sp.tile([128, 1024], F32, name="sc", tag="sc")
                    ex = expw.tile([128, 1024], F16, name="ex", tag="ex")
                    info = []
                    off = 0
                    for half in range(2):
                        j = 2 * p + half
                        tstart = max(128 * j, CCH * c)
                        w = CCH * (c + 1) - tstart
                        nc.tensor.matmul(
                            sc[0:128, off : off + w],
                            ctx["hk"][:, 128 * j : 128 * (j + 1)],
                            ctx["hq"][:, tstart - CCH * c : tstart - CCH * c + w],
                            start=True,
                            stop=True,
                        )
                        info.append((j, tstart, w, off))
                        off += w
                    nc.scalar.activation(
                        ex[:, 0:off],
                        sc[0:128, 0:off],
                        mybir.ActivationFunctionType.Exp,
                        scale=SCALE,
                    )
                    for j, tstart, w, o in info:
                        if 128 * j >= CCH * c:
                            nc.vector.tensor_mul(
                                out=ex[:, o : o + 128],
                                in0=ex[:, o : o + 128],
                                in1=mask[:],
                            )
                        lo = tstart - CCH * c
                        nc.tensor.matmul(
                            ctx["ytp_t"][0 : DH + 1, lo : lo + w],
                            vp[ctx["h"], j][:],
                            ex[:, o : o + w],
                            start=(j == 0),
                            stop=(j == jlast),
                        )

                def normalize(ctx):
                    # bc rides the sc pool's rotation (same slot size, no
                    # extra PSUM banks) -- keeping it out of the pp pool lets
                    # next-chunk projection matmuls fill attention gaps
                    m, b_, ytp_t = ctx["m"], ctx["b"], ctx["ytp_t"]
                    den = norm.tile([128, CCH], F32R, name="den", tag="den")
                    nc.vector.tensor_copy(out=den[64:65, :], in_=ytp_t[64:65, :])
                    bc = sp.tile([64, CCH], F32, name="bc", tag="sc")
                    nc.tensor.matmul(
                        bc[:], ones_r[64:65, :], den[64:65, :],
                        start=True, stop=True,
                    )
                    bcr = norm.tile([64, CCH], F32, name="bcr", tag="bcr")
                    nc.vector.reciprocal(bcr[:], bc[:])
                    if b_ == 0:
                        nc.vector.tensor_mul(
                            out=ysb[m][0:64, :], in0=ytp_t[0:DH, :], in1=bcr[:]
                        )
                    else:
                        ysh = norm.tile([64, CCH], F16, name="ysh", tag="ysh")
                        nc.vector.tensor_mul(
                            out=ysh[:], in0=ytp_t[0:DH, :], in1=bcr[:]
                        )
                        nc.gpsimd.dma_start(out=ysb[m][64:128, :], in_=ysh[:])

                for h in (1, 0, 3, 2):
                    ctx = head_ctx(h)
                    for p in range(2 * c + 2):
                        attend_pair(ctx, p)
                    normalize(ctx)
                return ysb

            def out_proj(c, ysb):
                """Partial out-projection (own 256 y-feats -> all 1024 outs).

                PSUM->SBUF(+bias) copies split across DVE (t=0) and the
                Activation engine (t=1, Identity+bias) so the post-attention
                epilogue runs on two engines in parallel; each half DMAs from
                its own HWDGE queue.
                """
                for t in range(4):
                    # 2 m-tiles per flush: one epilogue copy on DVE, one on
                    # the Act engine (parallel), then a quarter-DMA -- the
                    # last flush after the final matmul is short
                    ob = osb.tile([128, 2 * CCH], F16, name=f"ob{c}_{t}", tag="ob")
                    for s in range(2):
                        o = 2 * t + s
                        ps = pp.tile([128, CCH], F32, name="ops", tag="pp")
                        for m in range(2):
                            nc.tensor.matmul(
                                ps[:],
                                wo_sb[:, C * m + 128 * o : C * m + 128 * (o + 1)],
                                ysb[m][:],
                                start=(m == 0),
                                stop=(m == 1),
                            )
                        if s == 0:
                            nc.vector.tensor_scalar_add(
                                out=ob[:, CCH * s : CCH * (s + 1)],
                                in0=ps[:],
                                scalar1=bias_sb[:, o : o + 1],
                            )
                        else:
                            nc.scalar.activation(
                                ob[:, CCH * s : CCH * (s + 1)],
                                ps[:],
                                mybir.ActivationFunctionType.Identity,
                                bias=bias_sb[:, o : o + 1],
                            )
                    eng = nc.sync if t % 2 == 0 else nc.scalar
                    eng.dma_start(
                        out=partial[c]
                        .ap()[256 * t : 256 * (t + 1), :]
                        .rearrange("(k p) f -> p k f", p=128),
                        in_=ob[:].rearrange("p (k f) -> p k f", f=CCH),
                    )

            def reduce_scatter(c):
                nc.gpsimd.collective_compute(
                    "ReduceScatter",
                    mybir.AluOpType.add,
                    replica_groups=groups,
                    ins=[partial[c][:].opt()],
                    outs=[outr[c][:].opt()],
                )
                # chunks 0-2: copy on the Pool queue (already serialized on
                # the collectives); last chunk: split across the two idle
                # HWDGE queues so the final copy is fast
                if c < NCH - 1:
                    nc.sync.dma_start(out=outc[c][:, :], in_=outr[c][:, :])
                else:
                    nc.sync.dma_start(
                        out=outc[c][0:128, :], in_=outr[c][0:128, :]
                    )
                    nc.scalar.dma_start(
                        out=outc[c][128:256, :], in_=outr[c][128:256, :]
                    )

            # proj(c+1) is emitted BEFORE out_proj(c): its PSUM tiles then
            # rotate ahead of pout's in the pp pool, so the scheduler can
            # slot next-chunk projection matmuls into the PE gaps of the
            # Act-engine-paced attention stretch
            qT = project_chunk(0, xs_pre)
            for c in range(NCH):
                if c + 1 < NCH:
                    xss_next = prefetch_x(c + 1)
                ysb = attend_chunk(c, qT)
                if c + 1 < NCH:
                    qT = project_chunk(c + 1, xss_next)
                out_proj(c, ysb)
                reduce_scatter(c)

    return nc


_PROGRAM = None


def _get_program():
    global _PROGRAM
    if _PROGRAM is None:
        _apply_walrus_workaround()
        _PROGRAM = _build_program()
    return _PROGRAM


def kernel(x, w_qkv, w_out, b_out):
    x = np.asarray(x, dtype=np.float32)
    w_qkv = np.asarray(w_qkv, dtype=np.float32)
    w_out = np.asarray(w_out, dtype=np.float32)
    b_out = np.asarray(b_out, dtype=np.float32)

    bias_tile = np.ascontiguousarray(b_out.reshape(8, 128).T)  # [128, 8]
    zeros_tile = np.zeros_like(bias_tile)

    def sb_layout(wT):  # [1024or256, F] -> [128, (k f)] SBUF layout
        k = wT.shape[0] // 128
        return np.ascontiguousarray(
            wT.reshape(k, 128, -1).transpose(1, 0, 2).reshape(128, -1)
        )

    in_maps = []
    for i in range(N_CORES):
        b, g = divmod(i, 4)
        sl = slice(FEATS * g, FEATS * (g + 1))
        in_maps.append(
            {
                "xT": np.ascontiguousarray(x[b].T.astype(np.float16)),
                "wqT": sb_layout(w_qkv[0 * C :][sl].T.astype(np.float16)),
                "wkT": sb_layout(w_qkv[1 * C :][sl].T.astype(np.float16)),
                "wvT": sb_layout(w_qkv[2 * C :][sl].T.astype(np.float16)),
                "woT": sb_layout(w_out[:, sl].T.astype(np.float16)),
                "bo": bias_tile if g == 0 else zeros_tile,
            }
        )

    nc = _get_program()
    res = run_bass_kernel_spmd(nc, in_maps, core_ids=list(range(N_CORES)))
    kernel.last_results = res

    outs = []
    for b in range(B):
        full = np.empty((C, T), dtype=np.float32)
        for g in range(4):
            r = res.results[4 * b + g]
            for c in range(NCH):
                full[FEATS * g : FEATS * (g + 1), CCH * c : CCH * (c + 1)] = r[
                    f"outc{c}"
                ].astype(np.float32)
        outs.append(full.T)
    return np.stack(outs)


# revision 43
# speedup vs baseline: 1.0003x; 1.0003x over previous
"""Multi-head causal attention (B=2, T=2048, C=1024, H=16, Dh=64) on 8 TRN2 cores.

Sharding: batch x head tensor-parallel. Core i handles batch i//4 and heads
4*(i%4) .. 4*(i%4)+3. All weights and x are pre-transposed / fp16-converted on
the host, so the device does no layout work:
  1. per 512-token chunk, load xT columns and project qT/kT/v directly in the
     feature-on-partition layout (fp16 matmuls, fp32 PSUM),
  2. causal flash attention in scoresT (keys x tokens) layout; softmax
     denominators via a ones-column folded into v'; odd heads carry the ones
     column first so their denominator lands at partition 63 and their y rows
     at 64..127 -- every normalize op stays partition-aligned and two heads
     pack one [128, 512] tile,
  3. partial output projection over the core's own 256 y-features into all
     1024 output features (+bias on group-rank-0 cores, zeros elsewhere),
  4. per-chunk ReduceScatter (sum) over the 4 cores of the batch: each core
     receives its 256 output-feature rows, already fully reduced.
Host reassembles (concat feature shards per chunk, transpose to token-major).
"""

import json

import numpy as np

import concourse.bass as bass
import concourse.mybir as mybir
from concourse.tile import TileContext
from concourse.bass_utils import run_bass_kernel_spmd
from concourse.masks import make_identity, make_upper_triangular

F32 = mybir.dt.float32
F32R = mybir.dt.float32r
F16 = mybir.dt.float16

N_CORES = 8
B = 2
T = 2048          # tokens per batch (= per core)
C = 1024          # model dim
NH_CORE = 4       # heads per core
DH = 64
FEATS = NH_CORE * DH   # 256 per-core q/k/v features
CCH = 512         # attention t-chunk
NCH = T // CCH    # 4 chunks
KTILES = T // 128  # 16 k-tiles
SCALE = 1.0 / 8.0  # 1/sqrt(DH)


def _split_waits_in_bir(bir_bytes: bytes) -> bytes:
    """Workaround: installed walrus rejects >1 sync-wait per instruction."""
    bir = json.loads(bir_bytes)
    changed = False

    def rewrite(insts):
        nonlocal changed
        out = []
        for inst in insts:
            if isinstance(inst, dict):
                for v in inst.values():
                    visit(v)
                si = inst.get("sync_info")
                engine = inst.get("engine")
                if si and engine and len(si.get("on_wait") or []) > 1:
                    waits = si["on_wait"]
                    for i, w in enumerate(waits[:-1]):
                        out.append(
                            {
                                "debug": inst.get("debug", 0),
                                "engine": engine,
                                "ins": [],
                                "name": f"{inst['name']}_ws{i}",
                                "opcode": "EventSemaphore",
                                "outs": [],
                                "sync_info": {"on_update": [], "on_wait": [w]},
                            }
                        )
                    si["on_wait"] = [waits[-1]]
                    changed = True
            out.append(inst)
        insts[:] = out

    def visit(o):
        if isinstance(o, dict):
            for k, v in o.items():
                if k == "instructions" and isinstance(v, list):
                    rewrite(v)
                else:
                    visit(v)
        elif isinstance(o, list):
            for v in o:
                visit(v)

    visit(bir)
    return json.dumps(bir).encode() if changed else bir_bytes


_PATCHED = False


def _apply_walrus_workaround():
    global _PATCHED
    if _PATCHED:
        return
    import concourse.bass_utils as bass_utils
    import concourse.bass2jax as bass2jax

    orig = bass_utils.compile_bir_kernel

    def wrapped(bir_json, tmpdir, neff_name="file.neff"):
        return orig(_split_waits_in_bir(bir_json), tmpdir, neff_name)

    bass_utils.compile_bir_kernel = wrapped
    bass2jax.compile_bir_kernel = wrapped
    _PATCHED = True


def _build_program() -> bass.Bass:
    nc = bass.Bass(num_devices=N_CORES)

    xT = nc.dram_tensor("xT", [C, T], F16, kind="ExternalInput")
    # weight tensors arrive in SBUF layout: [128, 8*256] (col block k = wT
    # rows 128k..) / [128, 2*1024] (col block m = woT rows 128m..)
    wqT = nc.dram_tensor("wqT", [128, 8 * FEATS], F16, kind="ExternalInput")
    wkT = nc.dram_tensor("wkT", [128, 8 * FEATS], F16, kind="ExternalInput")
    wvT = nc.dram_tensor("wvT", [128, 8 * FEATS], F16, kind="ExternalInput")
    woT = nc.dram_tensor("woT", [128, 2 * C], F16, kind="ExternalInput")
    bo = nc.dram_tensor("bo", [128, 8], F32, kind="ExternalInput")

    partial = [nc.dram_tensor(f"partial{c}", [C, CCH], F16) for c in range(NCH)]
    outr = [nc.dram_tensor(f"outr{c}", [FEATS, CCH], F16) for c in range(NCH)]
    outc = [
        nc.dram_tensor(f"outc{c}", [FEATS, CCH], F16, kind="ExternalOutput")
        for c in range(NCH)
    ]
    groups = [[0, 1, 2, 3], [4, 5, 6, 7]]

    with TileContext(nc) as tc:
        with (
            tc.tile_pool(name="const", bufs=1) as cpool,
            tc.tile_pool(name="wts", bufs=1) as wpool,
            tc.tile_pool(name="xload", bufs=2) as xload,
            tc.tile_pool(name="qkv", bufs=1) as qkv,
            tc.tile_pool(name="qcur", bufs=2) as qcur,
            tc.tile_pool(name="vchunk", bufs=2) as vchunk,
            tc.tile_pool(name="expw", bufs=6) as expw,
            tc.tile_pool(name="norm", bufs=2) as norm,
            tc.tile_pool(name="ysbp", bufs=2) as ysbp,
            tc.tile_pool(name="osb", bufs=4) as osb,
            tc.tile_pool(name="pp", bufs=2, space="PSUM") as pp,
            tc.tile_pool(name="sp", bufs=2, space="PSUM") as sp,
            tc.tile_pool(name="yp", bufs=2, space="PSUM") as yp,
        ):
            # ---- startup loads, emitted in first-consumption order: the
            # modeled DMA device is serial, so transfer order ~= emission
            # order across the two HWDGE queues
            wsb = {}
            wsb["q"] = wpool.tile([128, 8 * FEATS], F16, name="w_q")
            xs_pre = []
            xb0t = [
                xload.tile([128, 4 * CCH], F16, name=f"xb0_{h}", tag=f"xb{h}")
                for h in range(2)
            ]
            for half in range(2):
                weng = nc.sync if half == 0 else nc.scalar
                xeng = nc.scalar if half == 0 else nc.sync
                weng.dma_start(
                    out=wsb["q"][:, 1024 * half : 1024 * (half + 1)],
                    in_=wqT[:, 1024 * half : 1024 * (half + 1)],
                )
                xeng.dma_start(
                    out=xb0t[half][:].rearrange("p (k f) -> p k f", f=CCH),
                    in_=xT.ap()[512 * half : 512 * (half + 1), 0:CCH]
                    .rearrange("(k p) f -> p k f", p=128),
                )
                for k in range(4):
                    xs_pre.append(xb0t[half][:, CCH * k : CCH * (k + 1)])
            for sec, wdram in (("k", wkT), ("v", wvT)):
                wt = wpool.tile([128, 8 * FEATS], F16, name=f"w_{sec}")
                for half in range(2):
                    eng = nc.sync if half == 0 else nc.scalar
                    eng.dma_start(
                        out=wt[:, 1024 * half : 1024 * (half + 1)],
                        in_=wdram[:, 1024 * half : 1024 * (half + 1)],
                    )
                wsb[sec] = wt
            wo_sb = wpool.tile([128, 2 * C], F16, name="w_o")
            for m in range(2):
                eng = nc.sync if m == 0 else nc.scalar
                eng.dma_start(
                    out=wo_sb[:, C * m : C * (m + 1)],
                    in_=woT[:, C * m : C * (m + 1)],
                )

            # ---- constants (needed from the v'-transpose / first exp on) ----
            identity = cpool.tile([128, 128], F16)
            make_identity(nc, identity[:])
            mask = cpool.tile([128, 128], F16)
            make_upper_triangular(nc, mask[:], val=1.0, diag=True)
            ones_r = cpool.tile([128, 64], F32R)
            nc.vector.memset(ones_r[:].bitcast(F32), 1.0)
            bias_sb = cpool.tile([128, 8], F32)
            nc.sync.dma_start(out=bias_sb[:], in_=bo[:, :])

            # ---- persistent activations ----
            kT = [qkv.tile([128, T], F16, name=f"kT_{m}") for m in range(2)]
            # v' tiles: [v | ones-col]; the ones row of the AV output is the
            # softmax denominator
            vp = {}
            for h in range(NH_CORE):
                for j in range(KTILES):
                    t = qkv.tile([128, DH + 1], F16, name=f"vp_{h}_{j}")
                    nc.vector.memset(t[:, DH : DH + 1], 1.0)
                    vp[h, j] = t

            def prefetch_x(n):
                t0 = CCH * n
                xss = []
                for half in range(2):
                    xb = xload.tile(
                        [128, 4 * CCH], F16, name=f"xb{n}_{half}", tag=f"xb{half}"
                    )
                    nc.scalar.dma_start(
                        out=xb[:].rearrange("p (k f) -> p k f", f=CCH),
                        in_=xT.ap()[512 * half : 512 * (half + 1), t0 : t0 + CCH]
                        .rearrange("(k p) f -> p k f", p=128),
                    )
                    for k in range(4):
                        xss.append(xb[:, CCH * k : CCH * (k + 1)])
                return xss

            def project_chunk(n, xss):
                """Project tokens [512n, 512n+512): qT/kT columns, v' tiles."""
                t0 = CCH * n
                qT = []
                vch = []
                for sec in ("q", "k", "v"):
                    for m in range(2):
                        ps = pp.tile([128, CCH], F32, name="projps", tag="pp")
                        for k in range(8):
                            nc.tensor.matmul(
                                ps[:],
                                wsb[sec][:, 256 * k + 128 * m : 256 * k + 128 * (m + 1)],
                                xss[k][:],
                                start=(k == 0),
                                stop=(k == 7),
                            )
                        if sec == "q":
                            qt = qkv.tile([128, CCH], F16, name=f"qT{n}_{m}")
                            nc.scalar.copy(out=qt[:], in_=ps[:])
                            qT.append(qt)
                        elif sec == "k":
                            nc.vector.tensor_copy(
                                out=kT[m][:, t0 : t0 + CCH], in_=ps[:]
                            )
                        else:
                            vc = vchunk.tile([128, CCH], F16, name=f"vc{n}_{m}", tag=f"vc{m}")
                            nc.vector.tensor_copy(out=vc[:], in_=ps[:])
                            vch.append(vc)
                for h in range(NH_CORE):
                    m, b_ = h // 2, h % 2
                    for jj in range(4):
                        j = 4 * n + jj
                        tp = pp.tile([128, DH], F16, name="vtr", tag="pp")
                        nc.tensor.matmul(
                            tp[:],
                            vch[m][64 * b_ : 64 * (b_ + 1), 128 * jj : 128 * (jj + 1)],
                            identity[64 * b_ : 64 * (b_ + 1), 64 * b_ : 64 * (b_ + 1)],
                            is_transpose=True,
                        )
                        nc.vector.tensor_copy(
                            out=vp[h, j][:, 0:DH], in_=tp[:]
                        )
                return qT

            def attend_chunk(c, qT):
                """Attention for tokens [512c, 512c+512), all heads + ysb."""
                jlast = 4 * c + 3
                ysb = [
                    ysbp.tile([128, CCH], F16, name=f"ysb{c}_{m}", tag=f"ysb{m}")
                    for m in range(2)
                ]
                # Heads run in interleaved PAIRS: while one head's exp is in
                # flight the PE streams the other head's matmuls, hiding the
                # score->exp->AV latency. Odd head leads (its ysb rows need a
                # partition-shifting SBUF->SBUF DMA, which then overlaps).
                def head_ctx(h):
                    m, b_ = h // 2, h % 2
                    ytp_t = yp.tile([128, CCH], F32, name=f"ytp{c}_{h}", tag="ytp")
                    return {
                        "h": h, "m": m, "b": b_,
                        "hq": qT[m][64 * b_ : 64 * (b_ + 1), :],
                        "hk": kT[m][64 * b_ : 64 * (b_ + 1), :],
                        "ytp_t": ytp_t,
                    }

                def attend_pair(ctx, p):
                    # two j-tiles share one PSUM score tile and one exp
                    # instruction: halves the Act engine's per-instruction
                    # access-latency overhead, which paces the pipeline
                    sc = sp.tile([128, 1024], F32, name="sc", tag="sc")
                    ex = expw.tile([128, 1024], F16, name="ex", tag="ex")
                    info = []
                    off = 0
                    for half in range(2):
                        j = 2 * p + half
                        tstart = max(128 * j, CCH * c)
                        w = CCH * (c + 1) - tstart
                        nc.tensor.matmul(
                            sc[0:128, off : off + w],
                            ctx["hk"][:, 128 * j : 128 * (j + 1)],
                            ctx["hq"][:, tstart - CCH * c : tstart - CCH * c + w],
                            start=True,
                            stop=True,
                        )
                        info.append((j, tstart, w, off))
                        off += w
                    nc.scalar.activation(
                        ex[:, 0:off],
                        sc[0:128, 0:off],
                        mybir.ActivationFunctionType.Exp,
                        scale=SCALE,
                    )
                    for j, tstart, w, o in info:
                        if 128 * j >= CCH * c:
                            nc.vector.tensor_mul(
                                out=ex[:, o : o + 128],
                                in0=ex[:, o : o + 128],
                                in1=mask[:],
                            )
                        lo = tstart - CCH * c
                        nc.tensor.matmul(
                            ctx["ytp_t"][0 : DH + 1, lo : lo + w],
                            vp[ctx["h"], j][:],
                            ex[:, o : o + w],
                            start=(j == 0),
                            stop=(j == jlast),
                        )

                def normalize(ctx):
                    # bc rides the sc pool's rotation (same slot size, no
                    # extra PSUM banks) -- keeping it out of the pp pool lets
                    # next-chunk projection matmuls fill attention gaps
                    m, b_, ytp_t = ctx["m"], ctx["b"], ctx["ytp_t"]
                    den = norm.tile([128, CCH], F32R, name="den", tag="den")
                    nc.vector.tensor_copy(out=den[64:65, :], in_=ytp_t[64:65, :])
                    bc = sp.tile([64, CCH], F32, name="bc", tag="sc")
                    nc.tensor.matmul(
                        bc[:], ones_r[64:65, :], den[64:65, :],
                        start=True, stop=True,
                    )
                    bcr = norm.tile([64, CCH], F32, name="bcr", tag="bcr")
                    nc.vector.reciprocal(bcr[:], bc[:])
                    if b_ == 0:
                        nc.vector.tensor_mul(
                            out=ysb[m][0:64, :], in0=ytp_t[0:DH, :], in1=bcr[:]
                        )
                    else:
                        ysh = norm.tile([64, CCH], F16, name="ysh", tag="ysh")
                        nc.vector.tensor_mul(
                            out=ysh[:], in0=ytp_t[0:DH, :], in1=bcr[:]
                        )
                        nc.gpsimd.dma_start(out=ysb[m][64:128, :], in_=ysh[:])

                for h in (1, 0, 3, 2):
                    ctx = head_ctx(h)
                    for p in range(2 * c + 2):
                        attend_pair(ctx, p)
                    normalize(ctx)
                return ysb

            def out_proj(c, ysb):
                """Partial out-projection (own 256 y-feats -> all 1024 outs).

                PSUM->SBUF(+bias) copies split across DVE (t=0) and the
                Activation engine (t=1, Identity+bias) so the post-attention
                epilogue runs on two engines in parallel; each half DMAs from
                its own HWDGE queue.
                """
                for t in range(4):
                    # 2 m-tiles per flush: one epilogue copy on DVE, one on
                    # the Act engine (parallel), then a quarter-DMA -- the
                    # last flush after the final matmul is short
                    ob = osb.tile([128, 2 * CCH], F16, name=f"ob{c}_{t}", tag="ob")
                    for s in range(2):
                        o = 2 * t + s
                        ps = pp.tile([128, CCH], F32, name="ops", tag="pp")
                        for m in range(2):
                            nc.tensor.matmul(
                                ps[:],
                                wo_sb[:, C * m + 128 * o : C * m + 128 * (o + 1)],
                                ysb[m][:],
                                start=(m == 0),
                                stop=(m == 1),
                            )
                        if s == 0:
                            nc.vector.tensor_scalar_add(
                                out=ob[:, CCH * s : CCH * (s + 1)],
                                in0=ps[:],
                                scalar1=bias_sb[:, o : o + 1],
                            )
                        else:
                            nc.scalar.activation(
                                ob[:, CCH * s : CCH * (s + 1)],
                                ps[:],
                                mybir.ActivationFunctionType.Identity,
                                bias=bias_sb[:, o : o + 1],
                            )
                    eng = nc.sync if t % 2 == 0 else nc.scalar
                    eng.dma_start(
                        out=partial[c]
                        .ap()[256 * t : 256 * (t + 1), :]
                        .rearrange("(k p) f -> p k f", p=128),
                        in_=ob[:].rearrange("p (k f) -> p k f", f=CCH),
                    )

            def reduce_scatter(c):
                nc.gpsimd.collective_compute(
                    "ReduceScatter",
                    mybir.AluOpType.add,
                    replica_groups=groups,
                    ins=[partial[c][:].opt()],
                    outs=[outr[c][:].opt()],
                )
                # chunks 0-2: copy on the Pool queue (already serialized on
                # the collectives); last chunk: split across the two idle
                # HWDGE queues so the final copy is fast
                if c < NCH - 1:
                    nc.sync.dma_start(out=outc[c][:, :], in_=outr[c][:, :])
                else:
                    nc.sync.dma_start(
                        out=outc[c][0:128, :], in_=outr[c][0:128, :]
                    )
                    nc.scalar.dma_start(
                        out=outc[c][128:256, :], in_=outr[c][128:256, :]
                    )

            # proj(c+1) is emitted BEFORE out_proj(c): its PSUM tiles then
            # rotate ahead of pout's in the pp pool, so the scheduler can
            # slot next-chunk projection matmuls into the PE gaps of the
            # Act-engine-paced attention stretch
            qT = project_chunk(0, xs_pre)
            for c in range(NCH):
                if c + 1 < NCH:
                    xss_next = prefetch_x(c + 1)
                ysb = attend_chunk(c, qT)
                if c + 1 < NCH:
                    qT = project_chunk(c + 1, xss_next)
                out_proj(c, ysb)
                reduce_scatter(c)

    return nc


_PROGRAM = None


def _get_program():
    global _PROGRAM
    if _PROGRAM is None:
        _apply_walrus_workaround()
        _PROGRAM = _build_program()
    return _PROGRAM


def kernel(x, w_qkv, w_out, b_out):
    x = np.asarray(x, dtype=np.float32)
    w_qkv = np.asarray(w_qkv, dtype=np.float32)
    w_out = np.asarray(w_out, dtype=np.float32)
    b_out = np.asarray(b_out, dtype=np.float32)

    bias_tile = np.ascontiguousarray(b_out.reshape(8, 128).T)  # [128, 8]
    zeros_tile = np.zeros_like(bias_tile)

    def sb_layout(wT):  # [1024or256, F] -> [128, (k f)] SBUF layout
        k = wT.shape[0] // 128
        return np.ascontiguousarray(
            wT.reshape(k, 128, -1).transpose(1, 0, 2).reshape(128, -1)
        )

    in_maps = []
    for i in range(N_CORES):
        b, g = divmod(i, 4)
        sl = slice(FEATS * g, FEATS * (g + 1))
        in_maps.append(
            {
                "xT": np.ascontiguousarray(x[b].T.astype(np.float16)),
                "wqT": sb_layout(w_qkv[0 * C :][sl].T.astype(np.float16)),
                "wkT": sb_layout(w_qkv[1 * C :][sl].T.astype(np.float16)),
                "wvT": sb_layout(w_qkv[2 * C :][sl].T.astype(np.float16)),
                "woT": sb_layout(w_out[:, sl].T.astype(np.float16)),
                "bo": bias_tile if g == 0 else zeros_tile,
            }
        )

    nc = _get_program()
    res = run_bass_kernel_spmd(nc, in_maps, core_ids=list(range(N_CORES)))
    kernel.last_results = res

    outs = []
    for b in range(B):
        full = np.empty((C, T), dtype=np.float32)
        for g in range(4):
            r = res.results[4 * b + g]
            for c in range(NCH):
                full[FEATS * g : FEATS * (g + 1), CCH * c : CCH * (c + 1)] = r[
                    f"outc{c}"
                ].astype(np.float32)
        outs.append(full.T)
    return np.stack(outs)


# revision 44
# speedup vs baseline: 1.0026x; 1.0023x over previous
"""Multi-head causal attention (B=2, T=2048, C=1024, H=16, Dh=64) on 8 TRN2 cores.

Sharding: batch x head tensor-parallel. Core i handles batch i//4 and heads
4*(i%4) .. 4*(i%4)+3. All weights and x are pre-transposed / fp16-converted on
the host, so the device does no layout work:
  1. per 512-token chunk, load xT columns and project qT/kT/v directly in the
     feature-on-partition layout (fp16 matmuls, fp32 PSUM),
  2. causal flash attention in scoresT (keys x tokens) layout; softmax
     denominators via a ones-column folded into v'; odd heads carry the ones
     column first so their denominator lands at partition 63 and their y rows
     at 64..127 -- every normalize op stays partition-aligned and two heads
     pack one [128, 512] tile,
  3. partial output projection over the core's own 256 y-features into all
     1024 output features (+bias on group-rank-0 cores, zeros elsewhere),
  4. per-chunk ReduceScatter (sum) over the 4 cores of the batch: each core
     receives its 256 output-feature rows, already fully reduced.
Host reassembles (concat feature shards per chunk, transpose to token-major).
"""

import json

import numpy as np

import concourse.bass as bass
import concourse.mybir as mybir
from concourse.tile import TileContext
from concourse.bass_utils import run_bass_kernel_spmd
from concourse.masks import make_identity, make_upper_triangular

F32 = mybir.dt.float32
F32R = mybir.dt.float32r
F16 = mybir.dt.float16

N_CORES = 8
B = 2
T = 2048          # tokens per batch (= per core)
C = 1024          # model dim
NH_CORE = 4       # heads per core
DH = 64
FEATS = NH_CORE * DH   # 256 per-core q/k/v features
CCH = 512         # attention t-chunk
NCH = T // CCH    # 4 chunks
KTILES = T // 128  # 16 k-tiles
SCALE = 1.0 / 8.0  # 1/sqrt(DH)


def _split_waits_in_bir(bir_bytes: bytes) -> bytes:
    """Workaround: installed walrus rejects >1 sync-wait per instruction."""
    bir = json.loads(bir_bytes)
    changed = False

    def rewrite(insts):
        nonlocal changed
        out = []
        for inst in insts:
            if isinstance(inst, dict):
                for v in inst.values():
                    visit(v)
                si = inst.get("sync_info")
                engine = inst.get("engine")
                if si and engine and len(si.get("on_wait") or []) > 1:
                    waits = si["on_wait"]
                    for i, w in enumerate(waits[:-1]):
                        out.append(
                            {
                                "debug": inst.get("debug", 0),
                                "engine": engine,
                                "ins": [],
                                "name": f"{inst['name']}_ws{i}",
                                "opcode": "EventSemaphore",
                                "outs": [],
                                "sync_info": {"on_update": [], "on_wait": [w]},
                            }
                        )
                    si["on_wait"] = [waits[-1]]
                    changed = True
            out.append(inst)
        insts[:] = out

    def visit(o):
        if isinstance(o, dict):
            for k, v in o.items():
                if k == "instructions" and isinstance(v, list):
                    rewrite(v)
                else:
                    visit(v)
        elif isinstance(o, list):
            for v in o:
                visit(v)

    visit(bir)
    return json.dumps(bir).encode() if changed else bir_bytes


_PATCHED = False


def _apply_walrus_workaround():
    global _PATCHED
    if _PATCHED:
        return
    import concourse.bass_utils as bass_utils
    import concourse.bass2jax as bass2jax

    orig = bass_utils.compile_bir_kernel

    def wrapped(bir_json, tmpdir, neff_name="file.neff"):
        return orig(_split_waits_in_bir(bir_json), tmpdir, neff_name)

    bass_utils.compile_bir_kernel = wrapped
    bass2jax.compile_bir_kernel = wrapped
    _PATCHED = True


def _build_program() -> bass.Bass:
    nc = bass.Bass(num_devices=N_CORES)

    xT = nc.dram_tensor("xT", [C, T], F16, kind="ExternalInput")
    # weight tensors arrive in SBUF layout: [128, 8*256] (col block k = wT
    # rows 128k..) / [128, 2*1024] (col block m = woT rows 128m..)
    wqT = nc.dram_tensor("wqT", [128, 8 * FEATS], F16, kind="ExternalInput")
    wkT = nc.dram_tensor("wkT", [128, 8 * FEATS], F16, kind="ExternalInput")
    wvT = nc.dram_tensor("wvT", [128, 8 * FEATS], F16, kind="ExternalInput")
    woT = nc.dram_tensor("woT", [128, 2 * C], F16, kind="ExternalInput")
    bo = nc.dram_tensor("bo", [128, 8], F32, kind="ExternalInput")

    partial = [nc.dram_tensor(f"partial{c}", [C, CCH], F16) for c in range(NCH)]
    outr = [nc.dram_tensor(f"outr{c}", [FEATS, CCH], F16) for c in range(NCH)]
    outc = [
        nc.dram_tensor(f"outc{c}", [FEATS, CCH], F16, kind="ExternalOutput")
        for c in range(NCH)
    ]
    groups = [[0, 1, 2, 3], [4, 5, 6, 7]]

    with TileContext(nc) as tc:
        with (
            tc.tile_pool(name="const", bufs=1) as cpool,
            tc.tile_pool(name="wts", bufs=1) as wpool,
            tc.tile_pool(name="xload", bufs=2) as xload,
            tc.tile_pool(name="qkv", bufs=1) as qkv,
            tc.tile_pool(name="qcur", bufs=2) as qcur,
            tc.tile_pool(name="vchunk", bufs=2) as vchunk,
            tc.tile_pool(name="expw", bufs=6) as expw,
            tc.tile_pool(name="norm", bufs=2) as norm,
            tc.tile_pool(name="ysbp", bufs=2) as ysbp,
            tc.tile_pool(name="osb", bufs=4) as osb,
            tc.tile_pool(name="pp", bufs=2, space="PSUM") as pp,
            tc.tile_pool(name="sp", bufs=2, space="PSUM") as sp,
            tc.tile_pool(name="yp", bufs=2, space="PSUM") as yp,
        ):
            # ---- startup loads, emitted in first-consumption order: the
            # modeled DMA device is serial, so transfer order ~= emission
            # order across the two HWDGE queues
            wsb = {}
            wsb["q"] = wpool.tile([128, 8 * FEATS], F16, name="w_q")
            xs_pre = []
            xb0t = [
                xload.tile([128, 4 * CCH], F16, name=f"xb0_{h}", tag=f"xb{h}")
                for h in range(2)
            ]
            for half in range(2):
                weng = nc.sync if half == 0 else nc.scalar
                xeng = nc.scalar if half == 0 else nc.sync
                weng.dma_start(
                    out=wsb["q"][:, 1024 * half : 1024 * (half + 1)],
                    in_=wqT[:, 1024 * half : 1024 * (half + 1)],
                )
                xeng.dma_start(
                    out=xb0t[half][:].rearrange("p (k f) -> p k f", f=CCH),
                    in_=xT.ap()[512 * half : 512 * (half + 1), 0:CCH]
                    .rearrange("(k p) f -> p k f", p=128),
                )
                for k in range(4):
                    xs_pre.append(xb0t[half][:, CCH * k : CCH * (k + 1)])
            for sec, wdram in (("k", wkT), ("v", wvT)):
                wt = wpool.tile([128, 8 * FEATS], F16, name=f"w_{sec}")
                for half in range(2):
                    eng = nc.sync if half == 0 else nc.scalar
                    eng.dma_start(
                        out=wt[:, 1024 * half : 1024 * (half + 1)],
                        in_=wdram[:, 1024 * half : 1024 * (half + 1)],
                    )
                wsb[sec] = wt
            wo_sb = wpool.tile([128, 2 * C], F16, name="w_o")
            for m in range(2):
                eng = nc.sync if m == 0 else nc.scalar
                eng.dma_start(
                    out=wo_sb[:, C * m : C * (m + 1)],
                    in_=woT[:, C * m : C * (m + 1)],
                )

            # ---- constants (needed from the v'-transpose / first exp on) ----
            identity = cpool.tile([128, 128], F16)
            make_identity(nc, identity[:])
            # PE warm-up: the p-state ramp needs ~3us of continuous PE busy
            # before full clock; burn it on dummy transposes during the
            # startup DMA wait so the first real matmuls run at 2.4GHz
            warm_in = cpool.tile([128, 128], F16)
            nc.vector.memset(warm_in[:], 0.0)
            warm_ps = pp.tile([128, 128], F16, name="warmps", tag="pp")
            for _ in range(56):
                nc.tensor.matmul(
                    warm_ps[:], warm_in[:], identity[:],
                    is_transpose=True, skip_group_check=True,
                )
            mask = cpool.tile([128, 128], F16)
            make_upper_triangular(nc, mask[:], val=1.0, diag=True)
            ones_r = cpool.tile([128, 64], F32R)
            nc.vector.memset(ones_r[:].bitcast(F32), 1.0)
            bias_sb = cpool.tile([128, 8], F32)
            nc.sync.dma_start(out=bias_sb[:], in_=bo[:, :])

            # ---- persistent activations ----
            kT = [qkv.tile([128, T], F16, name=f"kT_{m}") for m in range(2)]
            # v' tiles: [v | ones-col]; the ones row of the AV output is the
            # softmax denominator
            vp = {}
            for h in range(NH_CORE):
                for j in range(KTILES):
                    t = qkv.tile([128, DH + 1], F16, name=f"vp_{h}_{j}")
                    nc.vector.memset(t[:, DH : DH + 1], 1.0)
                    vp[h, j] = t

            def prefetch_x(n):
                t0 = CCH * n
                xss = []
                for half in range(2):
                    xb = xload.tile(
                        [128, 4 * CCH], F16, name=f"xb{n}_{half}", tag=f"xb{half}"
                    )
                    nc.scalar.dma_start(
                        out=xb[:].rearrange("p (k f) -> p k f", f=CCH),
                        in_=xT.ap()[512 * half : 512 * (half + 1), t0 : t0 + CCH]
                        .rearrange("(k p) f -> p k f", p=128),
                    )
                    for k in range(4):
                        xss.append(xb[:, CCH * k : CCH * (k + 1)])
                return xss

            def project_chunk(n, xss):
                """Project tokens [512n, 512n+512): qT/kT columns, v' tiles."""
                t0 = CCH * n
                qT = []
                vch = []
                for sec in ("q", "k", "v"):
                    for m in range(2):
                        ps = pp.tile([128, CCH], F32, name="projps", tag="pp")
                        for k in range(8):
                            nc.tensor.matmul(
                                ps[:],
                                wsb[sec][:, 256 * k + 128 * m : 256 * k + 128 * (m + 1)],
                                xss[k][:],
                                start=(k == 0),
                                stop=(k == 7),
                            )
                        if sec == "q":
                            qt = qkv.tile([128, CCH], F16, name=f"qT{n}_{m}")
                            nc.scalar.copy(out=qt[:], in_=ps[:])
                            qT.append(qt)
                        elif sec == "k":
                            nc.vector.tensor_copy(
                                out=kT[m][:, t0 : t0 + CCH], in_=ps[:]
                            )
                        else:
                            vc = vchunk.tile([128, CCH], F16, name=f"vc{n}_{m}", tag=f"vc{m}")
                            nc.vector.tensor_copy(out=vc[:], in_=ps[:])
                            vch.append(vc)
                for h in range(NH_CORE):
                    m, b_ = h // 2, h % 2
                    for jj in range(4):
                        j = 4 * n + jj
                        tp = pp.tile([128, DH], F16, name="vtr", tag="pp")
                        nc.tensor.matmul(
                            tp[:],
                            vch[m][64 * b_ : 64 * (b_ + 1), 128 * jj : 128 * (jj + 1)],
                            identity[64 * b_ : 64 * (b_ + 1), 64 * b_ : 64 * (b_ + 1)],
                            is_transpose=True,
                        )
                        nc.vector.tensor_copy(
                            out=vp[h, j][:, 0:DH], in_=tp[:]
                        )
                return qT

            def attend_chunk(c, qT):
                """Attention for tokens [512c, 512c+512), all heads + ysb."""
                jlast = 4 * c + 3
                ysb = [
                    ysbp.tile([128, CCH], F16, name=f"ysb{c}_{m}", tag=f"ysb{m}")
                    for m in range(2)
                ]
                # Heads run in interleaved PAIRS: while one head's exp is in
                # flight the PE streams the other head's matmuls, hiding the
                # score->exp->AV latency. Odd head leads (its ysb rows need a
                # partition-shifting SBUF->SBUF DMA, which then overlaps).
                def head_ctx(h):
                    m, b_ = h // 2, h % 2
                    ytp_t = yp.tile([128, CCH], F32, name=f"ytp{c}_{h}", tag="ytp")
                    return {
                        "h": h, "m": m, "b": b_,
                        "hq": qT[m][64 * b_ : 64 * (b_ + 1), :],
                        "hk": kT[m][64 * b_ : 64 * (b_ + 1), :],
                        "ytp_t": ytp_t,
                    }

                def attend_pair(ctx, p):
                    # two j-tiles share one PSUM score tile and one exp
                    # instruction: halves the Act engine's per-instruction
                    # access-latency overhead, which paces the pipeline
                    sc = sp.tile([128, 1024], F32, name="sc", tag="sc")
                    ex = expw.tile([128, 1024], F16, name="ex", tag="ex")
                    info = []
                    off = 0
                    for half in range(2):
                        j = 2 * p + half
                        tstart = max(128 * j, CCH * c)
                        w = CCH * (c + 1) - tstart
                        nc.tensor.matmul(
                            sc[0:128, off : off + w],
                            ctx["hk"][:, 128 * j : 128 * (j + 1)],
                            ctx["hq"][:, tstart - CCH * c : tstart - CCH * c + w],
                            start=True,
                            stop=True,
                        )
                        info.append((j, tstart, w, off))
                        off += w
                    nc.scalar.activation(
                        ex[:, 0:off],
                        sc[0:128, 0:off],
                        mybir.ActivationFunctionType.Exp,
                        scale=SCALE,
                    )
                    for j, tstart, w, o in info:
                        if 128 * j >= CCH * c:
                            nc.vector.tensor_mul(
                                out=ex[:, o : o + 128],
                                in0=ex[:, o : o + 128],
                                in1=mask[:],
                            )
                        lo = tstart - CCH * c
                        nc.tensor.matmul(
                            ctx["ytp_t"][0 : DH + 1, lo : lo + w],
                            vp[ctx["h"], j][:],
                            ex[:, o : o + w],
                            start=(j == 0),
                            stop=(j == jlast),
                        )

                def normalize(ctx):
                    # bc rides the sc pool's rotation (same slot size, no
                    # extra PSUM banks) -- keeping it out of the pp pool lets
                    # next-chunk projection matmuls fill attention gaps
                    m, b_, ytp_t = ctx["m"], ctx["b"], ctx["ytp_t"]
                    den = norm.tile([128, CCH], F32R, name="den", tag="den")
                    nc.vector.tensor_copy(out=den[64:65, :], in_=ytp_t[64:65, :])
                    bc = sp.tile([64, CCH], F32, name="bc", tag="sc")
                    nc.tensor.matmul(
                        bc[:], ones_r[64:65, :], den[64:65, :],
                        start=True, stop=True,
                    )
                    bcr = norm.tile([64, CCH], F32, name="bcr", tag="bcr")
                    nc.vector.reciprocal(bcr[:], bc[:])
                    if b_ == 0:
                        nc.vector.tensor_mul(
                            out=ysb[m][0:64, :], in0=ytp_t[0:DH, :], in1=bcr[:]
                        )
                    else:
                        ysh = norm.tile([64, CCH], F16, name="ysh", tag="ysh")
                        nc.vector.tensor_mul(
                            out=ysh[:], in0=ytp_t[0:DH, :], in1=bcr[:]
                        )
                        nc.gpsimd.dma_start(out=ysb[m][64:128, :], in_=ysh[:])

                for h in (1, 0, 3, 2):
                    ctx = head_ctx(h)
                    for p in range(2 * c + 2):
                        attend_pair(ctx, p)
                    normalize(ctx)
                return ysb

            def out_proj(c, ysb):
                """Partial out-projection (own 256 y-feats -> all 1024 outs).

                PSUM->SBUF(+bias) copies split across DVE (t=0) and the
                Activation engine (t=1, Identity+bias) so the post-attention
                epilogue runs on two engines in parallel; each half DMAs from
                its own HWDGE queue.
                """
                for t in range(4):
                    # 2 m-tiles per flush: one epilogue copy on DVE, one on
                    # the Act engine (parallel), then a quarter-DMA -- the
                    # last flush after the final matmul is short
                    ob = osb.tile([128, 2 * CCH], F16, name=f"ob{c}_{t}", tag="ob")
                    for s in range(2):
                        o = 2 * t + s
                        ps = pp.tile([128, CCH], F32, name="ops", tag="pp")
                        for m in range(2):
                            nc.tensor.matmul(
                                ps[:],
                                wo_sb[:, C * m + 128 * o : C * m + 128 * (o + 1)],
                                ysb[m][:],
                                start=(m == 0),
                                stop=(m == 1),
                            )
                        if s == 0:
                            nc.vector.tensor_scalar_add(
                                out=ob[:, CCH * s : CCH * (s + 1)],
                                in0=ps[:],
                                scalar1=bias_sb[:, o : o + 1],
                            )
                        else:
                            nc.scalar.activation(
                                ob[:, CCH * s : CCH * (s + 1)],
                                ps[:],
                                mybir.ActivationFunctionType.Identity,
                                bias=bias_sb[:, o : o + 1],
                            )
                    eng = nc.sync if t % 2 == 0 else nc.scalar
                    eng.dma_start(
                        out=partial[c]
                        .ap()[256 * t : 256 * (t + 1), :]
                        .rearrange("(k p) f -> p k f", p=128),
                        in_=ob[:].rearrange("p (k f) -> p k f", f=CCH),
                    )

            def reduce_scatter(c):
                nc.gpsimd.collective_compute(
                    "ReduceScatter",
                    mybir.AluOpType.add,
                    replica_groups=groups,
                    ins=[partial[c][:].opt()],
                    outs=[outr[c][:].opt()],
                )
                # chunks 0-2: copy on the Pool queue (already serialized on
                # the collectives); last chunk: split across the two idle
                # HWDGE queues so the final copy is fast
                if c < NCH - 1:
                    nc.sync.dma_start(out=outc[c][:, :], in_=outr[c][:, :])
                else:
                    nc.sync.dma_start(
                        out=outc[c][0:128, :], in_=outr[c][0:128, :]
                    )
                    nc.scalar.dma_start(
                        out=outc[c][128:256, :], in_=outr[c][128:256, :]
                    )

            # proj(c+1) is emitted BEFORE out_proj(c): its PSUM tiles then
            # rotate ahead of pout's in the pp pool, so the scheduler can
            # slot next-chunk projection matmuls into the PE gaps of the
            # Act-engine-paced attention stretch
            qT = project_chunk(0, xs_pre)
            for c in range(NCH):
                if c + 1 < NCH:
                    xss_next = prefetch_x(c + 1)
                ysb = attend_chunk(c, qT)
                if c + 1 < NCH:
                    qT = project_chunk(c + 1, xss_next)
                out_proj(c, ysb)
                reduce_scatter(c)

    return nc


_PROGRAM = None


def _get_program():
    global _PROGRAM
    if _PROGRAM is None:
        _apply_walrus_workaround()
        _PROGRAM = _build_program()
    return _PROGRAM


def kernel(x, w_qkv, w_out, b_out):
    x = np.asarray(x, dtype=np.float32)
    w_qkv = np.asarray(w_qkv, dtype=np.float32)
    w_out = np.asarray(w_out, dtype=np.float32)
    b_out = np.asarray(b_out, dtype=np.float32)

    bias_tile = np.ascontiguousarray(b_out.reshape(8, 128).T)  # [128, 8]
    zeros_tile = np.zeros_like(bias_tile)

    def sb_layout(wT):  # [1024or256, F] -> [128, (k f)] SBUF layout
        k = wT.shape[0] // 128
        return np.ascontiguousarray(
            wT.reshape(k, 128, -1).transpose(1, 0, 2).reshape(128, -1)
        )

    in_maps = []
    for i in range(N_CORES):
        b, g = divmod(i, 4)
        sl = slice(FEATS * g, FEATS * (g + 1))
        in_maps.append(
            {
                "xT": np.ascontiguousarray(x[b].T.astype(np.float16)),
                "wqT": sb_layout(w_qkv[0 * C :][sl].T.astype(np.float16)),
                "wkT": sb_layout(w_qkv[1 * C :][sl].T.astype(np.float16)),
                "wvT": sb_layout(w_qkv[2 * C :][sl].T.astype(np.float16)),
                "woT": sb_layout(w_out[:, sl].T.astype(np.float16)),
                "bo": bias_tile if g == 0 else zeros_tile,
            }
        )

    nc = _get_program()
    res = run_bass_kernel_spmd(nc, in_maps, core_ids=list(range(N_CORES)))
    kernel.last_results = res

    outs = []
    for b in range(B):
        full = np.empty((C, T), dtype=np.float32)
        for g in range(4):
            r = res.results[4 * b + g]
            for c in range(NCH):
                full[FEATS * g : FEATS * (g + 1), CCH * c : CCH * (c + 1)] = r[
                    f"outc{c}"
                ].astype(np.float32)
        outs.append(full.T)
    return np.stack(outs)
